# revision 1
# baseline (speedup 1.0000x reference)
"""Trainium2 kernel for nn_LoRALinear (moe_routing).

Math: reference computes out = x @ W.T + einsum('bri,bro->bo', a, b) with
a = A_table[dom].reshape(B,R,IN), b = B_table[dom].reshape(B,R,OUT).
The einsum contracts i over `a` alone, so the LoRA term collapses to a
per-domain table:
    L[d, o] = sum_r (sum_i A_table[d].reshape(R,IN)[r,i]) * B_table[d].reshape(R,OUT)[r,o]
    out = x @ W.T + L[domain_id]

On device this is a single augmented matmul per batch row:
    out[m, :] = [x[m, :], onehot(dom[m])] @ [[W.T], [L]]
with contraction K = 1024 (8 chunks of 128) plus a K=64 one-hot chunk.
The one-hot rows select L rows exactly (0/1 are exact in bf16). The two
K=64 one-hot matmuls per m-tile are packed into disjoint PE row groups
(tile_position) so they run concurrently.

Sharding: data-parallel over batch across 8 cores; the augmented weight is
replicated.

Device layout: the host pre-transposes activations into chunk-major form
xa[p, mb, k, j] = xaT[k*128 + p, mb*MB + j] so each m-block is a single
contiguous-per-partition DMA covering all 9 K-chunks (chunk 8 carries the
one-hot rows duplicated into both half-partitions).
"""

import functools

import numpy as np

import concourse.mybir as mybir
import concourse.tile as tile
from concourse import bacc, bass_utils

B, D, R, ND = 16384, 1024, 8, 64
N_CORES = 8
BS = B // N_CORES            # 2048 batch rows per core
NKW = 8                      # K chunks of 128 for the dense W part
NK = NKW + 1                 # + one-hot chunk
MB = 512                     # batch rows per x block DMA
NMB = BS // MB               # 4 blocks
OH = 512                     # psum free dim (one bank)


@functools.lru_cache(maxsize=1)
def _build():
    nc = bacc.Bacc(None, target_bir_lowering=False, debug=False)
    bf16 = mybir.dt.bfloat16
    xa = nc.dram_tensor("xa", [128, NMB * NK * MB], bf16, kind="ExternalInput")
    wa = nc.dram_tensor("wa", [NKW * 128, D], bf16, kind="ExternalInput")
    # L table packed for row-group concurrency: rows 0:64 = L[:, 0:512],
    # rows 64:128 = L[:, 512:1024]
    w8 = nc.dram_tensor("w8", [128, OH], bf16, kind="ExternalInput")
    out = nc.dram_tensor("out", [BS, D], mybir.dt.float32, kind="ExternalOutput")

    with tile.TileContext(nc) as tc:
        with (
            tc.tile_pool(name="w", bufs=1) as wpool,
            tc.tile_pool(name="x", bufs=2) as xpool,
            tc.tile_pool(name="o", bufs=4) as opool,
            tc.tile_pool(name="ps", bufs=7, space="PSUM") as pspool,
            tc.tile_pool(name="dps", bufs=1, space="PSUM") as dpspool,
        ):
            # Warm the PE (HAM clock gate) with dummy matmuls on a scratch
            # tile while the first DMAs stream in; otherwise the first ~12
            # real matmuls run at half clock.
            scratch = wpool.tile([128, OH], bf16, tag="scratch")
            nc.gpsimd.memset(scratch[:], 0.0)
            dps = dpspool.tile([128, OH], mybir.dt.float32, tag="dps")
            for i in range(12):
                nc.tensor.matmul(
                    dps[:],
                    scratch[:, 0:128],
                    scratch[:],
                    start=(i == 0),
                    stop=(i == 11),
                )

            # x block 0 first so its transfer overlaps the W preload.
            xts = {}
            xt0 = xpool.tile([128, NK * MB], bf16, tag="x")
            nc.sync.dma_start(xt0[:], xa[:, 0 : NK * MB])
            xts[0] = xt0

            wts = []
            for k in range(NKW):
                wt = wpool.tile([128, D], bf16, tag=f"w{k}")
                nc.sync.dma_start(wt[:], wa[k * 128 : (k + 1) * 128, :])
                wts.append(wt)
            w8t = wpool.tile([128, OH], bf16, tag="w8")
            nc.sync.dma_start(w8t[:], w8[:, :])

            def xsl(xt, k, mt):
                return xt[:, k * MB + mt * 128 : k * MB + (mt + 1) * 128]

            def finish(xt, mt, pss, mb):
                """One-hot row-group-packed matmuls + psum copies + out DMA."""
                nc.tensor.matmul(
                    pss[0][:],
                    xt[0:64, NKW * MB + mt * 128 : NKW * MB + (mt + 1) * 128],
                    w8t[0:64, :],
                    start=False,
                    stop=True,
                    tile_position=(0, 0),
                )
                nc.tensor.matmul(
                    pss[1][:],
                    xt[64:128, NKW * MB + mt * 128 : NKW * MB + (mt + 1) * 128],
                    w8t[64:128, :],
                    start=False,
                    stop=True,
                    tile_position=(64, 0),
                )
                ot = opool.tile([128, D], mybir.dt.float32, tag="ot")
                nc.vector.tensor_copy(ot[:, 0:OH], pss[0][:])
                nc.scalar.copy(ot[:, OH : 2 * OH], pss[1][:])
                m0 = mb * MB + mt * 128
                nc.sync.dma_start(out[m0 : m0 + 128, :], ot[:])

            # First two m-tiles: k-interleaved across 4 psum groups so each
            # arriving W chunk immediately feeds 4 matmuls (keeps the PE fed
            # while W streams in).
            pss = {}
            for g in range(4):
                psg = pspool.tile([128, OH], mybir.dt.float32, tag="ps")
                pss[g] = psg
            for k in range(NKW):
                for g in range(4):
                    mt, oh = divmod(g, 2)
                    nc.tensor.matmul(
                        pss[g][:],
                        xsl(xt0, k, mt),
                        wts[k][:, oh * OH : (oh + 1) * OH],
                        start=(k == 0),
                        stop=False,
                    )
            finish(xt0, 0, (pss[0], pss[1]), 0)
            finish(xt0, 1, (pss[2], pss[3]), 0)

            for mb in range(NMB):
                if mb not in xts:
                    xtn = xpool.tile([128, NK * MB], bf16, tag="x")
                    nc.sync.dma_start(
                        xtn[:], xa[:, mb * NK * MB : (mb + 1) * NK * MB]
                    )
                    xts[mb] = xtn
                xt = xts[mb]
                for mt in range(MB // 128):
                    if mb == 0 and mt < 2:
                        continue  # handled by the k-interleaved prologue
                    ps0 = pspool.tile([128, OH], mybir.dt.float32, tag="ps")
                    ps1 = pspool.tile([128, OH], mybir.dt.float32, tag="ps")
                    for k in range(NKW):
                        nc.tensor.matmul(
                            ps0[:],
                            xsl(xt, k, mt),
                            wts[k][:, 0:OH],
                            start=(k == 0),
                            stop=False,
                        )
                    for k in range(NKW):
                        nc.tensor.matmul(
                            ps1[:],
                            xsl(xt, k, mt),
                            wts[k][:, OH : 2 * OH],
                            start=(k == 0),
                            stop=False,
                        )
                    finish(xt, mt, (ps0, ps1), mb)

    nc.compile()
    return nc


def _prepare(x, W, A_table, B_table, domain_id):
    import ml_dtypes

    bf16 = np.dtype(ml_dtypes.bfloat16)
    x = np.asarray(x, dtype=np.float32)
    W = np.asarray(W, dtype=np.float32)
    A = np.asarray(A_table, dtype=np.float64)
    Bt = np.asarray(B_table, dtype=np.float64)
    dom = np.asarray(domain_id).astype(np.int64)

    sA = A.reshape(ND, R, D).sum(axis=2)                        # [ND, R]
    L = np.einsum("dr,dro->do", sA, Bt.reshape(ND, R, D))       # [ND, D]
    Lb = L.astype(np.float32).astype(bf16)

    wa = np.ascontiguousarray(W.T.astype(bf16))                 # [D, D]
    w8 = np.empty((128, OH), dtype=bf16)
    w8[0:ND] = Lb[:, 0:OH]
    w8[ND : 2 * ND] = Lb[:, OH : 2 * OH]

    xT = np.ascontiguousarray(x.T).astype(bf16)                 # [D, B]
    onehotT = (
        np.arange(ND, dtype=np.int64)[:, None] == dom[None, :]
    ).astype(bf16)                                              # [ND, B]

    in_maps = []
    for c in range(N_CORES):
        sl = slice(c * BS, (c + 1) * BS)
        xaT_c = np.empty((NK * 128, BS), dtype=bf16)
        xaT_c[: NKW * 128] = xT[:, sl]
        xaT_c[NKW * 128 : NKW * 128 + ND] = onehotT[:, sl]
        xaT_c[NKW * 128 + ND :] = onehotT[:, sl]                # duplicate
        # chunk-major: xa[p, mb, k, j] = xaT_c[k*128 + p, mb*MB + j]
        xa_c = np.ascontiguousarray(
            xaT_c.reshape(NK, 128, NMB, MB).transpose(1, 2, 0, 3)
        ).reshape(128, NMB * NK * MB)
        in_maps.append({"xa": xa_c, "wa": wa, "w8": w8})
    return in_maps


def kernel(x, W, A_table, B_table, domain_id, _trace=False):
    in_maps = _prepare(x, W, A_table, B_table, domain_id)
    nc = _build()
    res = bass_utils.run_bass_kernel_spmd(
        nc, in_maps, core_ids=list(range(N_CORES)), trace=_trace
    )
    out = np.concatenate([res.results[c]["out"] for c in range(N_CORES)], axis=0)
    if _trace:
        kernel.last_results = res
    return out



# revision 2
# speedup vs baseline: 1.0562x; 1.0562x over previous
"""Trainium2 kernel for nn_LoRALinear (moe_routing).

Math: reference computes out = x @ W.T + einsum('bri,bro->bo', a, b) with
a = A_table[dom].reshape(B,R,IN), b = B_table[dom].reshape(B,R,OUT).
The einsum contracts i over `a` alone, so the LoRA term collapses to a
per-domain table:
    L[d, o] = sum_r (sum_i A_table[d].reshape(R,IN)[r,i]) * B_table[d].reshape(R,OUT)[r,o]
    out = x @ W.T + L[domain_id]

Device work per core (data-parallel over batch, 2048 rows/core):
    out_tile[mt] = x[mt] @ W.T + Lg[mt]
where Lg = L[domain_id] is gathered on the host (a 64x1024 table lookup)
and streamed alongside x. The dense matmul runs as 16 m-tiles x 8 k-chunks
x 2 n-halves of [128x128] @ [128x512] bf16 MMs with the x block stationary
(LDWEIGHTS overlaps in-flight MMs via the background weight buffer, so the
PE streams at the 512-cycle/MM peak). The LoRA add rides the PSUM->SBUF
drain as a DVE tensor_add, so no partial-row-group matmuls are needed.

Outputs are written bf16 (host upcasts) to halve the store traffic; input
loads go on the sync HWDGE queue and stores on the scalar queue so they
don't head-of-line block each other.
"""

import functools

import numpy as np

import concourse.mybir as mybir
import concourse.tile as tile
from concourse import bacc, bass_utils

B, D, R, ND = 16384, 1024, 8, 64
N_CORES = 8
BS = B // N_CORES            # 2048 batch rows per core
NK = 8                       # k chunks of 128
NMT = BS // 128              # 16 m-tiles per core
MTW = 2 * D                  # xaug cols per m-tile: 1024 x-chunks + 1024 Lg

# m-tiles per DMA block: small blocks first so compute starts early
X_PLAN = [[0], [1], [2, 3], [4, 5, 6], [7, 8, 9], [10, 11, 12], [13, 14, 15]]


@functools.lru_cache(maxsize=1)
def _build():
    nc = bacc.Bacc(None, target_bir_lowering=False, debug=False)
    bf16 = mybir.dt.bfloat16
    f32 = mybir.dt.float32
    xa = nc.dram_tensor("xa", [128, NMT * MTW], bf16, kind="ExternalInput")
    wa = nc.dram_tensor("wa", [128, NK * D], bf16, kind="ExternalInput")
    out = nc.dram_tensor("out", [128, NMT * D], bf16, kind="ExternalOutput")

    with tile.TileContext(nc) as tc:
        with (
            tc.tile_pool(name="w", bufs=1) as wpool,
            tc.tile_pool(name="x", bufs=1) as xpool,
            tc.tile_pool(name="o", bufs=2) as opool,
            tc.tile_pool(name="ps", bufs=3, space="PSUM") as pspool,
            tc.tile_pool(name="dps", bufs=1, space="PSUM") as dpspool,
        ):
            # Warm the PE (HAM clock gate) with dummy matmuls while the
            # first DMAs stream in.
            scratch = wpool.tile([128, 512], bf16, tag="scratch")
            nc.gpsimd.memset(scratch[:], 0.0)
            dps = dpspool.tile([128, 512], f32, tag="dps")
            for i in range(14):
                nc.tensor.matmul(
                    dps[:], scratch[:, 0:128], scratch[:],
                    start=(i == 0), stop=(i == 13),
                )

            wts = []
            xtiles = {}

            def dma_w(j):
                wt = wpool.tile([128, 2 * D], bf16, tag=f"w{j}")
                nc.sync.dma_start(wt[:], wa[:, j * 2 * D : (j + 1) * 2 * D])
                wts.append(wt)

            def dma_x(g):
                mts = X_PLAN[g]
                t = xpool.tile([128, len(mts) * MTW], bf16, tag=f"x{g}")
                nc.sync.dma_start(
                    t[:], xa[:, mts[0] * MTW : (mts[-1] + 1) * MTW]
                )
                for i, mt in enumerate(mts):
                    xtiles[mt] = (t, i * MTW)

            # issue order = consumption order
            dma_w(0)
            dma_w(1)
            dma_x(0)
            dma_w(2)
            dma_w(3)
            for g in range(1, len(X_PLAN)):
                dma_x(g)

            ot = None
            for mt in range(NMT):
                xt, xof = xtiles[mt]
                ps = pspool.tile([128, 2 * 512], f32, tag="ps")
                for k in range(NK):
                    wt = wts[k // 2]
                    wof = (k % 2) * D
                    lhsT = xt[:, xof + k * 128 : xof + (k + 1) * 128]
                    nc.tensor.matmul(
                        ps[:, 0:512], lhsT, wt[:, wof : wof + 512],
                        start=(k == 0), stop=(k == NK - 1),
                    )
                    nc.tensor.matmul(
                        ps[:, 512:1024], lhsT, wt[:, wof + 512 : wof + D],
                        start=(k == 0), stop=(k == NK - 1),
                    )
                if mt % 2 == 0:
                    ot = opool.tile([128, 2 * D], bf16, tag="ot")
                nc.vector.tensor_add(
                    ot[:, (mt % 2) * D : (mt % 2 + 1) * D],
                    ps[:],
                    xt[:, xof + D : xof + 2 * D],
                )
                if mt % 2 == 1:
                    nc.scalar.dma_start(
                        out[:, (mt - 1) * D : (mt + 1) * D], ot[:]
                    )

    nc.compile()
    return nc


def _prepare(x, W, A_table, B_table, domain_id):
    import ml_dtypes

    bf16 = np.dtype(ml_dtypes.bfloat16)
    x = np.asarray(x, dtype=np.float32)
    W = np.asarray(W, dtype=np.float32)
    A = np.asarray(A_table, dtype=np.float64)
    Bt = np.asarray(B_table, dtype=np.float64)
    dom = np.asarray(domain_id).astype(np.int64)

    sA = A.reshape(ND, R, D).sum(axis=2)                        # [ND, R]
    L = np.einsum("dr,dro->do", sA, Bt.reshape(ND, R, D))       # [ND, D]
    Lg = L.astype(np.float32)[dom].astype(bf16)                 # [B, D]

    # W.T chunk-major: wa[p, k*D + n] = W.T[k*128+p, n]
    wa = np.ascontiguousarray(
        W.T.astype(bf16).reshape(NK, 128, D).transpose(1, 0, 2)
    ).reshape(128, NK * D)

    in_maps = []
    for c in range(N_CORES):
        sl = slice(c * BS, (c + 1) * BS)
        xc = x[sl].astype(bf16)                                 # [2048, 1024]
        # xpart[p, mt, k*128+j] = xc[mt*128+j, k*128+p]
        xpart = xc.reshape(NMT, 128, NK, 128).transpose(3, 0, 2, 1)
        lgpart = Lg[sl].reshape(NMT, 128, D).transpose(1, 0, 2)  # [p, mt, n]
        xaug = np.empty((128, NMT, MTW), dtype=bf16)
        xaug[:, :, 0:D] = xpart.reshape(128, NMT, D)
        xaug[:, :, D:MTW] = lgpart
        in_maps.append({"xa": xaug.reshape(128, NMT * MTW), "wa": wa})
    return in_maps


def kernel(x, W, A_table, B_table, domain_id, _trace=False):
    in_maps = _prepare(x, W, A_table, B_table, domain_id)
    nc = _build()
    res = bass_utils.run_bass_kernel_spmd(
        nc, in_maps, core_ids=list(range(N_CORES)), trace=_trace
    )
    outs = []
    for c in range(N_CORES):
        oc = res.results[c]["out"]                              # [128, NMT*D] bf16
        outs.append(
            oc.reshape(128, NMT, D)
            .transpose(1, 0, 2)
            .reshape(BS, D)
            .astype(np.float32)
        )
    out = np.concatenate(outs, axis=0)
    if _trace:
        kernel.last_results = res
    return out
